# revision 2
# baseline (speedup 1.0000x reference)
"""Trainium2 Bass kernel: CaputoFractionalActivation (tanh base, alpha=0.5, 10 terms).

Math: the reference evaluates tanh at 11 points x - k*h (h in [1e-6, 1e-3]) and
takes the Caputo finite-difference series.  Because h is tiny, the series
collapses (Taylor around x, S0 = sum_j w_j = 0 exactly) to

    out = t - (1 - t^2) * (S1 + S2 * h * t) + O(h^2),   t = tanh(x)

The h-dependent part is bounded by |S2|*h_max*max|t(1-t^2)| ~ 1.4e-2 absolute
(5.4e-3 of the 1.62 output scale) and contributes only ~4.4e-3 relative l2 --
far inside the 2e-2 gate -- so this kernel drops it entirely:

    out = t + S1 * (t^2 - 1)

which is a pure elementwise function of x: no global min/max, no collective.
Measured vs the f32 reference: l2 rel err 4.4e-3, absmax 1.1e-2 (both
deterministic -- the reference input is a fixed seed).

The input is sent to the device as fp16 (1 ulp of x costs < 2.3e-4 in tanh,
negligible here), halving input HBM traffic: 4 MiB in + 4 MiB out per core
= 8.4 MB, a ~23 us DMA floor per core at ~360 GB/s.

Device program per core (data-parallel shard of x over 8 cores, [128, 16384]
fp16 per core), tiles sized [2048, 2048, 4096, 4096, 4096] with two schemes
balanced so ACT ~19 us and DVE ~19 us both sit under the DMA floor:
- scheme A (ACT-heavy): t = Tanh(x); y = Square(sqrt(S1)*t) = S1*t^2 [ACT];
  o = (y - S1) + t  [one DVE scalar_tensor_tensor]
- scheme B (DVE-heavy): t = Tanh(x) [ACT]; q = t*t [TT]; m = S1*q - S1
  [dual-scalar TS]; o = m + t [TT]
All intermediates fp16 (DVE 2x/4x perf modes); output fp16, widened on host.
"""

import math

import numpy as np

import concourse.bacc as bacc
import concourse.mybir as mybir
from concourse import tile
from concourse.bass_utils import run_bass_kernel_spmd

N_CORES = 8
ALPHA = 0.5
N_TERMS = 10
_COEF = [
    ((-1.0) ** k) * math.gamma(ALPHA + k + 1.0) / (math.factorial(k) * math.gamma(ALPHA + 1.0))
    for k in range(N_TERMS)
]
S1 = -sum(_COEF)          # 1.4535369873046866
SQS = math.sqrt(S1)       # Square(SQS*t) = S1*t^2

# Full input (4, 4096, 1024) f32, sharded 8 ways on axis 1 -> (4, 512, 1024)
# per core = 2,097,152 elements = [128 partitions, 16384 free] fp16.
B, T, D = 4, 4096, 1024
P = 128
F = (B * T * D) // (N_CORES * P)  # 16384

# Tile schedule: small tiles first for pipeline ramp, then 1 MiB tiles.
# 'A' = ACT Square + 1 DVE op; 'B' = tanh only on ACT + 3 DVE ops.
TILES = [2048, 2048, 4096, 4096, 4096]
SCHEMES = ["B", "B", "A", "B", "B"]
assert sum(TILES) == F


def emit(nc, x_d, o_d, sfx=""):
    """Emit the per-core program. x_d/o_d: [P, F] fp16 DRAM APs."""
    with tile.TileContext(nc) as tc:
        emit_in_tc(tc, x_d, o_d, sfx=sfx)


def emit_in_tc(tc, x_d, o_d, sfx=""):
    nc = tc.nc
    fp16 = mybir.dt.float16
    AT = mybir.AluOpType
    AF = mybir.ActivationFunctionType
    fdmax = max(TILES)

    with (
        tc.tile_pool(name="xin" + sfx, bufs=3) as px,
        tc.tile_pool(name="tan" + sfx, bufs=3) as pt,
        tc.tile_pool(name="tmp" + sfx, bufs=4) as pm,
        tc.tile_pool(name="out" + sfx, bufs=3) as po,
        tc.tile_pool(name="sml" + sfx, bufs=1) as ps,
    ):
        # tiny warmup activation so the ACT func-table load (~2.7 us)
        # overlaps the first DMA instead of gating the first real tanh
        warm = ps.tile([P, 1], fp16, tag="warm")
        nc.vector.memset(warm[:], 0.0)
        nc.scalar.activation(warm[:], warm[:], AF.Tanh)

        off = 0
        for i, fd in enumerate(TILES):
            sl = slice(off, off + fd)
            off += fd
            xt = px.tile([P, fdmax], fp16, tag="xin")
            nc.sync.dma_start(xt[:, :fd], x_d[:, sl])
            t = pt.tile([P, fdmax], fp16, tag="tan")
            nc.scalar.activation(t[:, :fd], xt[:, :fd], AF.Tanh)
            o = po.tile([P, fdmax], fp16, tag="out")
            if SCHEMES[i] == "A":
                y = pm.tile([P, fdmax], fp16, tag="tmp")
                nc.scalar.activation(y[:, :fd], t[:, :fd], AF.Square, scale=SQS)
                nc.vector.scalar_tensor_tensor(
                    o[:, :fd], y[:, :fd], S1, t[:, :fd], AT.subtract, AT.add
                )
            else:
                q = pm.tile([P, fdmax], fp16, tag="tmp")
                nc.vector.tensor_tensor(q[:, :fd], t[:, :fd], t[:, :fd], AT.mult)
                m = pm.tile([P, fdmax], fp16, tag="tmp")
                nc.vector.tensor_scalar(m[:, :fd], q[:, :fd], S1, S1, AT.mult, AT.subtract)
                nc.vector.tensor_tensor(o[:, :fd], m[:, :fd], t[:, :fd], AT.add)
            nc.sync.dma_start(o_d[:, sl], o[:, :fd])


def build(reps=1):
    nc = bacc.Bacc("TRN2", target_bir_lowering=False, debug=False, num_devices=N_CORES)
    fp16 = mybir.dt.float16
    x_d = nc.dram_tensor("x", [P, F], fp16, kind="ExternalInput").ap()
    o_d = nc.dram_tensor("out", [P, F], fp16, kind="ExternalOutput").ap()
    if reps == 0:
        # near-empty program for launch-overhead calibration
        with tile.TileContext(nc) as tc:
            with tc.tile_pool(name="cal", bufs=1) as pc:
                tcal = pc.tile([1, 2], fp16, tag="cal")
                nc.sync.dma_start(tcal[:], x_d[:1, :2])
                nc.sync.dma_start(o_d[:1, :2], tcal[:])
    for r in range(reps):
        emit(nc, x_d, o_d, sfx=f"_r{r}")
    nc.compile()
    return nc


_NC_CACHE = {}


def make_in_maps(x):
    """Shard full (4, 4096, 1024) f32 -> 8 x [P, F] fp16 device inputs."""
    ts = T // N_CORES
    return [
        {"x": x[:, i * ts : (i + 1) * ts, :].astype(np.float16).reshape(P, F)}
        for i in range(N_CORES)
    ]


def run(x, trace=False, **kw):
    """x: full (4, 4096, 1024) f32. Returns (full_out, BassKernelResults)."""
    key = "nc"
    if key not in _NC_CACHE:
        _NC_CACHE[key] = build()
    nc = _NC_CACHE[key]
    in_maps = make_in_maps(x)
    # Transient device wedges (NRT_EXEC_UNIT_UNRECOVERABLE) have been observed
    # to clear after ~30-60 s; retry with backoff.
    import time as _time

    br = None
    for attempt, delay in enumerate((0, 30, 60)):
        if delay:
            _time.sleep(delay)
        try:
            br = run_bass_kernel_spmd(
                nc, in_maps, core_ids=list(range(N_CORES)), trace=trace, **kw
            )
            break
        except Exception:
            if attempt == 2:
                raise
    ts = T // N_CORES
    shards = [
        br.results[i]["out"].astype(np.float32).reshape(B, ts, D)
        for i in range(N_CORES)
    ]
    out = np.concatenate(shards, axis=1)
    return out, br


def kernel(**inputs):
    x = np.asarray(inputs["x"], dtype=np.float32)
    out, _ = run(x)
    return out.astype(np.float32)


# revision 15
# speedup vs baseline: 1.0057x; 1.0057x over previous
"""Trainium2 Bass kernel: CaputoFractionalActivation (tanh base, alpha=0.5, 10 terms).

Math: the reference evaluates tanh at 11 points x - k*h (h in [1e-6, 1e-3]) and
takes the Caputo finite-difference series.  Because h is tiny, the series
collapses (Taylor around x, S0 = sum_j w_j = 0 exactly) to

    out = t - (1 - t^2) * (S1 + S2 * h * t) + O(h^2),   t = tanh(x)

The h-dependent part is bounded by |S2|*h_max*max|t(1-t^2)| ~ 1.4e-2 absolute
(5.4e-3 of the 1.62 output scale) and contributes only ~4.4e-3 relative l2 --
far inside the 2e-2 gate -- so this kernel drops it entirely:

    out = t + S1 * (t^2 - 1)

which is a pure elementwise function of x: no global min/max, no collective.
Measured vs the f32 reference: l2 rel err 4.4e-3, absmax 1.1e-2 (both
deterministic -- the reference input is a fixed seed).

The input is sent to the device as fp16 (1 ulp of x costs < 2.3e-4 in tanh,
negligible here), halving input HBM traffic: 4 MiB in + 4 MiB out per core
= 8.4 MB, a ~23 us DMA floor per core at ~360 GB/s.

Device program per core (data-parallel shard of x over 8 cores, [128, 16384]
fp16 per core), tiles sized [2048, 2048, 4096, 4096, 4096] with two schemes
balanced so ACT ~19 us and DVE ~19 us both sit under the DMA floor:
- scheme A (ACT-heavy): t = Tanh(x); y = Square(sqrt(S1)*t) = S1*t^2 [ACT];
  o = (y - S1) + t  [one DVE scalar_tensor_tensor]
- scheme B (DVE-heavy): t = Tanh(x) [ACT]; q = t*t [TT]; m = S1*q - S1
  [dual-scalar TS]; o = m + t [TT]
All intermediates fp16 (DVE 2x/4x perf modes); output fp16, widened on host.
"""

import math

import numpy as np

import concourse.bacc as bacc
import concourse.mybir as mybir
from concourse import tile
from concourse.bass_utils import run_bass_kernel_spmd

N_CORES = 8
ALPHA = 0.5
N_TERMS = 10
_COEF = [
    ((-1.0) ** k) * math.gamma(ALPHA + k + 1.0) / (math.factorial(k) * math.gamma(ALPHA + 1.0))
    for k in range(N_TERMS)
]
S1 = -sum(_COEF)          # 1.4535369873046866
SQS = math.sqrt(S1)       # Square(SQS*t) = S1*t^2
C0 = 1.0 / (2.0 * S1)
SQB = SQS * C0            # Square(SQS*t + SQB) = S1*(t+C0)^2 = t + S1*t^2 + S1*C0^2
CZZ = -(S1 * C0 * C0 + S1)  # ... + CZZ = t + S1*(t^2 - 1)

# Full input (4, 4096, 1024) f32, sharded 8 ways on axis 1 -> (4, 512, 1024)
# per core = 2,097,152 elements = [128 partitions, 16384 free] fp16.
B, T, D = 4, 4096, 1024
P = 128
F = (B * T * D) // (N_CORES * P)  # 16384

# DMA chunking is decoupled from compute tiling: IN_CHUNKS/OUT_CHUNKS set the
# dma_start granularity; COMPUTE_TILES sets the ACT/DVE instruction
# granularity (chunk boundaries must align with compute-tile boundaries).
# Schemes per compute tile:
#   'A' = ACT Square + 1 DVE scalar_tensor_tensor (ACT-heavy)
#   'B' = 2 DVE scalar_tensor_tensor (1x mode -- slow, kept for A/B)
#   'C' = DVE tensor_tensor + dual-scalar tensor_scalar + tensor_tensor
#         (2x/4x modes -- fastest DVE path)
IN_CHUNKS = [512, 1536] + [2048] * 6 + [1024, 1024]
OUT_CHUNKS = [512, 1536] + [2048] * 6 + [1024, 1024]
COMPUTE_TILES = [512, 1536] + [2048] * 6 + [1024, 1024]
SCHEMES = list("CCCCCCCDDD")
# Prefetch all input DMAs before the compute loop so the SP sequencer's
# in-order issue of out-DMAs (each waiting on its chunk's DVE) cannot delay
# later input loads.
PREFETCH = True
assert sum(IN_CHUNKS) == F and sum(OUT_CHUNKS) == F
assert sum(COMPUTE_TILES) == F


def emit(nc, x_d, o_d, sfx=""):
    """Emit the per-core program. x_d/o_d: [P, F] fp16 DRAM APs."""
    with tile.TileContext(nc) as tc:
        emit_in_tc(tc, x_d, o_d, sfx=sfx)


def emit_in_tc(tc, x_d, o_d, sfx=""):
    nc = tc.nc
    fp16 = mybir.dt.float16
    AT = mybir.AluOpType
    AF = mybir.ActivationFunctionType

    in_offs = [sum(IN_CHUNKS[:i]) for i in range(len(IN_CHUNKS))]
    out_offs = [sum(OUT_CHUNKS[:i]) for i in range(len(OUT_CHUNKS))]
    with (
        tc.tile_pool(name="xin" + sfx, bufs=1) as px,
        tc.tile_pool(name="tan" + sfx, bufs=3) as pt,
        tc.tile_pool(name="tmp" + sfx, bufs=4) as pm,
        tc.tile_pool(name="out" + sfx, bufs=1) as po,
        tc.tile_pool(name="sml" + sfx, bufs=1) as ps,
    ):
        # tiny warmup activation so the ACT func-table load (~2.7 us)
        # overlaps the first DMA instead of gating the first real tanh
        warm = ps.tile([P, 1], fp16, tag="warm")
        nc.vector.memset(warm[:], 0.0)
        nc.scalar.activation(warm[:], warm[:], AF.Tanh)
        sqb = None
        if "D" in SCHEMES:
            sqb = ps.tile([P, 1], mybir.dt.float32, tag="sqb")
            nc.vector.memset(sqb[:], SQB)

        # prefetch all input chunks (SP issues these before any out-DMA wait)
        inbufs = []
        for ci, cs in enumerate(IN_CHUNKS):
            xt = px.tile([P, cs], fp16, tag=f"xin{ci}", name=f"xin{ci}" + sfx)
            if PREFETCH:
                nc.sync.dma_start(xt[:], x_d[:, in_offs[ci] : in_offs[ci] + cs])
            inbufs.append(xt)
        outbufs = [
            po.tile([P, cs], fp16, tag=f"out{ci}", name=f"out{ci}" + sfx)
            for ci, cs in enumerate(OUT_CHUNKS)
        ]

        def views(goff, ct, offs, sizes, bufs):
            """SBUF view of [goff, goff+ct) inside its chunk buffer."""
            for ci in range(len(sizes)):
                if offs[ci] <= goff < offs[ci] + sizes[ci]:
                    lo = goff - offs[ci]
                    assert lo + ct <= sizes[ci]
                    return ci, bufs[ci][:, lo : lo + ct]
            raise AssertionError

        goff = 0
        for i, ct in enumerate(COMPUTE_TILES):
            ici, xv = views(goff, ct, in_offs, IN_CHUNKS, inbufs)
            if not PREFETCH and goff == in_offs[ici]:
                nc.sync.dma_start(
                    inbufs[ici][:], x_d[:, in_offs[ici] : in_offs[ici] + IN_CHUNKS[ici]]
                )
            oci, ov = views(goff, ct, out_offs, OUT_CHUNKS, outbufs)
            t = pt.tile([P, ct], fp16, tag=f"tan{ct}")
            nc.scalar.activation(t[:], xv, AF.Tanh)
            if SCHEMES[i] == "A":
                y = pm.tile([P, ct], fp16, tag=f"tmp{ct}")
                nc.scalar.activation(y[:], t[:], AF.Square, scale=SQS)
                nc.vector.scalar_tensor_tensor(ov, y[:], S1, t[:], AT.subtract, AT.add)
            elif SCHEMES[i] == "D":
                # y = S1*(t+C0)^2 = t + S1*t^2 + S1*C0^2; o = y + CZZ
                y = pm.tile([P, ct], fp16, tag=f"tmp{ct}")
                nc.scalar.activation(y[:], t[:], AF.Square, bias=sqb[:], scale=SQS)
                nc.vector.tensor_scalar(ov, y[:], CZZ, None, AT.add)
            elif SCHEMES[i] == "C":
                q = pm.tile([P, ct], fp16, tag=f"tmp{ct}")
                nc.vector.tensor_tensor(q[:], t[:], t[:], AT.mult)
                m = pm.tile([P, ct], fp16, tag=f"tmp{ct}")
                nc.vector.tensor_scalar(m[:], q[:], S1, S1, AT.mult, AT.subtract)
                nc.vector.tensor_tensor(ov, m[:], t[:], AT.add)
            else:
                q = pm.tile([P, ct], fp16, tag=f"tmp{ct}")
                nc.vector.scalar_tensor_tensor(q[:], t[:], S1, t[:], AT.mult, AT.mult)
                nc.vector.scalar_tensor_tensor(ov, q[:], S1, t[:], AT.subtract, AT.add)
            goff += ct
            # last compute sub-tile of this out-chunk -> drain it
            if goff == out_offs[oci] + OUT_CHUNKS[oci]:
                nc.sync.dma_start(
                    o_d[:, out_offs[oci] : out_offs[oci] + OUT_CHUNKS[oci]],
                    outbufs[oci][:],
                )


def build(reps=1):
    nc = bacc.Bacc("TRN2", target_bir_lowering=False, debug=False, num_devices=N_CORES)
    fp16 = mybir.dt.float16
    x_d = nc.dram_tensor("x", [P, F], fp16, kind="ExternalInput").ap()
    o_d = nc.dram_tensor("out", [P, F], fp16, kind="ExternalOutput").ap()
    if reps == 0:
        # near-empty program for launch-overhead calibration
        with tile.TileContext(nc) as tc:
            with tc.tile_pool(name="cal", bufs=1) as pc:
                tcal = pc.tile([1, 2], fp16, tag="cal")
                nc.sync.dma_start(tcal[:], x_d[:1, :2])
                nc.sync.dma_start(o_d[:1, :2], tcal[:])
    for r in range(reps):
        emit(nc, x_d, o_d, sfx=f"_r{r}")
    nc.compile()
    return nc


_NC_CACHE = {}


def make_in_maps(x):
    """Shard full (4, 4096, 1024) f32 -> 8 x [P, F] fp16 device inputs."""
    ts = T // N_CORES
    return [
        {"x": x[:, i * ts : (i + 1) * ts, :].astype(np.float16).reshape(P, F)}
        for i in range(N_CORES)
    ]


def run(x, trace=False, **kw):
    """x: full (4, 4096, 1024) f32. Returns (full_out, BassKernelResults)."""
    key = "nc"
    if key not in _NC_CACHE:
        _NC_CACHE[key] = build()
    nc = _NC_CACHE[key]
    in_maps = make_in_maps(x)
    # Transient device wedges (NRT_EXEC_UNIT_UNRECOVERABLE) have been observed
    # to clear after ~30-60 s; retry with backoff.
    import time as _time

    br = None
    for attempt, delay in enumerate((0, 30, 60)):
        if delay:
            _time.sleep(delay)
        try:
            br = run_bass_kernel_spmd(
                nc, in_maps, core_ids=list(range(N_CORES)), trace=trace, **kw
            )
            break
        except Exception:
            if attempt == 2:
                raise
    ts = T // N_CORES
    shards = [
        br.results[i]["out"].astype(np.float32).reshape(B, ts, D)
        for i in range(N_CORES)
    ]
    out = np.concatenate(shards, axis=1)
    return out, br


def kernel(**inputs):
    x = np.asarray(inputs["x"], dtype=np.float32)
    out, _ = run(x)
    return out.astype(np.float32)
